# revision 1
# baseline (speedup 1.0000x reference)
"""Causal single-head attention on 8 trn2 NeuronCores.

Problem (hardcoded): x [256,256,384] f32, Wq/Wk/Wv [384,64] f32
  q,k,v = x@W;  S = q@k^T * 384**-0.5; causal softmax; out = P@v  [256,256,64]

Sharding: data-parallel over batch B=256 -> 32 batches per core; weights
replicated. Per batch (T=256 tokens, C=384, H=64), per core:

  1. DMA x_b [256,384] as two [128,384] tiles (t-chunks).
  2. PE-transpose (fp32, exact) 6 128x128 blocks -> x^T [384c, 256t] in SBUF
     (rounded to f32r by the PSUM->SBUF evacuation copies).
  3. kT/qT = Wk^T@x^T, Wq^T@x^T   [64,256] each (f32r matmuls, N=256)
     vT = Wv^T@x^T [64,256]; append ones row -> v'T [65,256]; PE-transpose to
     v' [128,65] per t-chunk (v natural + ones column).
  4. S^T[j,i] per j-chunk: lhsT=kT chunk, rhs=qT  -> [128,256] PSUM.
     P^T = exp(scale*S^T) via ACT (PSUM->SBUF, f32r), then causal mask:
     multiplicative 0/1 upper-triangular 128x128 tile (+ zeroing the
     all-masked left half of chunk 1). No max-subtraction: |scale*S| <~ 3.
  5. O'^T [65,256] = sum_j v'[j,:]^T... accumulated over both j-chunks.
     Row 64 = softmax denominators (ones row of v').
  6. PE-transpose O'^T back to [128,65] per t-chunk; normalize cols 0:64 by
     reciprocal of col 64; DMA out.
"""
import numpy as np

N_CORES = 8
B, T, C, H = 256, 256, 384, 64
NB = B // N_CORES          # 32 batches per core
SCALE = float(C) ** -0.5

_state = {}


def _build():
    import concourse.bacc as bacc
    import concourse.tile as tile
    import concourse.mybir as mybir
    from concourse.masks import make_identity, make_upper_triangular

    dt = mybir.dt
    f32 = dt.float32
    f32r = dt.float32r
    AF = mybir.ActivationFunctionType

    nc = bacc.Bacc("TRN2", target_bir_lowering=False)
    x_d = nc.dram_tensor("x", [NB, T, C], f32, kind="ExternalInput")
    wq_d = nc.dram_tensor("Wq", [C, H], f32, kind="ExternalInput")
    wk_d = nc.dram_tensor("Wk", [C, H], f32, kind="ExternalInput")
    wv_d = nc.dram_tensor("Wv", [C, H], f32, kind="ExternalInput")
    out_d = nc.dram_tensor("out", [NB, T, H], f32, kind="ExternalOutput")

    with tile.TileContext(nc) as tc:
        with tc.tile_pool(name="setup", bufs=1) as setup, \
             tc.tile_pool(name="xin", bufs=3) as xin, \
             tc.tile_pool(name="work", bufs=3) as work, \
             tc.tile_pool(name="ps", bufs=1, space="PSUM") as ps:

            # --- one-time setup ---
            ident = setup.tile([128, 128], f32)
            make_identity(nc, ident)
            mask_st = setup.tile([128, 128], f32)
            make_upper_triangular(nc, mask_st, val=1.0, diag=True)
            mask = setup.tile([128, 128], f32r)
            nc.vector.tensor_copy(mask, mask_st)

            w_stage = setup.tile([128, 3 * C // 128 * 0 + 576], f32)  # [128, 576]
            # cc-chunk cc occupies cols cc*192:(cc+1)*192 as [Wk|Wq|Wv]
            for cc in range(3):
                nc.sync.dma_start(out=w_stage[:, cc * 192 + 0: cc * 192 + 64],
                                  in_=wk_d[cc * 128:(cc + 1) * 128, :])
                nc.sync.dma_start(out=w_stage[:, cc * 192 + 64: cc * 192 + 128],
                                  in_=wq_d[cc * 128:(cc + 1) * 128, :])
                nc.sync.dma_start(out=w_stage[:, cc * 192 + 128: cc * 192 + 192],
                                  in_=wv_d[cc * 128:(cc + 1) * 128, :])
            w_all = setup.tile([128, 576], f32r)
            nc.vector.tensor_copy(w_all, w_stage)  # round to f32r

            def wslice(cc, which):  # which: 0=k 1=q 2=v
                lo = cc * 192 + which * 64
                return w_all[:, lo:lo + 64]

            # --- per-batch pipeline ---
            for b in range(NB):
                x0 = xin.tile([128, C], f32)
                x1 = xin.tile([128, C], f32)
                nc.sync.dma_start(out=x0, in_=x_d[b, 0:128, :])
                nc.sync.dma_start(out=x1, in_=x_d[b, 128:256, :])

                # transpose x -> x^T  (xtps_a holds cc0+cc1, xtps_b holds cc2)
                xtps_a = ps.tile([128, 512], f32)
                xtps_b = ps.tile([128, 256], f32)
                for cc in range(3):
                    dst = xtps_a if cc < 2 else xtps_b
                    base = (cc % 2) * 256 if cc < 2 else 0
                    nc.tensor.transpose(dst[:, base:base + 128],
                                        x0[:, cc * 128:(cc + 1) * 128], ident)
                    nc.tensor.transpose(dst[:, base + 128:base + 256],
                                        x1[:, cc * 128:(cc + 1) * 128], ident)
                xt = work.tile([128, 768], f32r)
                nc.scalar.copy(xt[:, 0:512], xtps_a)
                nc.vector.tensor_copy(xt[:, 512:768], xtps_b)

                def xts(cc):
                    return xt[:, cc * 256:(cc + 1) * 256]

                # kT / qT  -> one PSUM bank [64, 512]
                kqps = ps.tile([64, 512], f32)
                for cc in range(3):
                    nc.tensor.matmul(kqps[:, 0:256], wslice(cc, 0), xts(cc),
                                     start=(cc == 0), stop=(cc == 2))
                for cc in range(3):
                    nc.tensor.matmul(kqps[:, 256:512], wslice(cc, 1), xts(cc),
                                     start=(cc == 0), stop=(cc == 2))
                kq_k = work.tile([64, 256], f32r)
                kq_q = work.tile([64, 256], f32r)
                nc.vector.tensor_copy(kq_k, kqps[:, 0:256])
                nc.scalar.copy(kq_q, kqps[:, 256:512])

                # vT [64,256] -> v'T [65,256] (ones row) -> v' [128,65] per tc
                vtps = ps.tile([64, 256], f32)
                for cc in range(3):
                    nc.tensor.matmul(vtps, wslice(cc, 2), xts(cc),
                                     start=(cc == 0), stop=(cc == 2))
                vtp = work.tile([65, 256], f32)
                nc.scalar.copy(vtp[0:64, :], vtps)
                nc.gpsimd.memset(vtp[64:65, :], 1.0)
                vpps = ps.tile([128, 130], f32)
                vp = work.tile([128, 130], f32r)
                nc.tensor.transpose(vpps[:, 0:65], vtp[:, 0:128],
                                    ident[0:65, 0:65])
                nc.tensor.transpose(vpps[:, 65:130], vtp[:, 128:256],
                                    ident[0:65, 0:65])
                nc.vector.tensor_copy(vp, vpps)
                vp0 = vp[:, 0:65]
                vp1 = vp[:, 65:130]

                # S^T per j-chunk + exp + causal mask
                stps = ps.tile([128, 512], f32)
                nc.tensor.matmul(stps[:, 0:256], kq_k[:, 0:128], kq_q,
                                 start=True, stop=True)
                nc.tensor.matmul(stps[:, 256:512], kq_k[:, 128:256], kq_q,
                                 start=True, stop=True)
                pt0 = work.tile([128, 256], f32r)
                pt1 = work.tile([128, 128], f32r)
                nc.scalar.activation(pt0, stps[:, 0:256], AF.Exp, scale=SCALE)
                # chunk-1 rows attend only to keys j>=128 -> cols 128:256
                nc.scalar.activation(pt1, stps[:, 384:512], AF.Exp, scale=SCALE)
                nc.vector.tensor_mul(pt0[:, 0:128], pt0[:, 0:128], mask)
                nc.vector.tensor_mul(pt1, pt1, mask)

                # O'^T [65,256] accumulate over j-chunks (chunk 1 only touches
                # output cols 128:256; cols 0:128 get no chunk-1 contribution)
                ops = ps.tile([65, 256], f32)
                nc.tensor.matmul(ops, vp0, pt0, start=True, stop=False)
                nc.tensor.matmul(ops[:, 128:256], vp1, pt1,
                                 start=False, stop=True)
                ot = work.tile([65, 256], f32)
                nc.vector.tensor_copy(ot, ops)

                # transpose back, normalize, store
                ofps = ps.tile([128, 130], f32)
                nc.tensor.transpose(ofps[:, 0:65], ot[:, 0:128],
                                    ident[0:65, 0:65])
                nc.tensor.transpose(ofps[:, 65:130], ot[:, 128:256],
                                    ident[0:65, 0:65])
                rec0 = work.tile([128, 1], f32)
                rec1 = work.tile([128, 1], f32)
                nc.vector.reciprocal(rec0, ofps[:, 64:65])
                nc.vector.reciprocal(rec1, ofps[:, 129:130])
                oo0 = work.tile([128, 64], f32)
                oo1 = work.tile([128, 64], f32)
                nc.vector.tensor_scalar_mul(oo0, ofps[:, 0:64], rec0)
                nc.scalar.mul(oo1, ofps[:, 65:129], rec1)
                nc.sync.dma_start(out=out_d[b, 0:128, :], in_=oo0)
                nc.sync.dma_start(out=out_d[b, 128:256, :], in_=oo1)

    nc.finalize()
    return nc


def kernel(x, Wq, Wk, Wv, _trace=False):
    from concourse.bass_utils import run_bass_kernel_spmd

    if "nc" not in _state:
        _state["nc"] = _build()
    nc = _state["nc"]

    x = np.ascontiguousarray(np.asarray(x, dtype=np.float32))
    wq = np.ascontiguousarray(np.asarray(Wq, dtype=np.float32))
    wk = np.ascontiguousarray(np.asarray(Wk, dtype=np.float32))
    wv = np.ascontiguousarray(np.asarray(Wv, dtype=np.float32))

    in_maps = [
        {"x": x[i * NB:(i + 1) * NB], "Wq": wq, "Wk": wk, "Wv": wv}
        for i in range(N_CORES)
    ]
    res = run_bass_kernel_spmd(nc, in_maps, core_ids=list(range(N_CORES)),
                               trace=_trace)
    _state["exec_time_ns"] = res.exec_time_ns
    _state["trace"] = res.instructions_and_trace
    return np.concatenate([res.results[i]["out"] for i in range(N_CORES)],
                          axis=0)



# revision 27
# speedup vs baseline: 2.2353x; 2.2353x over previous
"""Causal single-head attention on 8 trn2 NeuronCores.

Problem (hardcoded): x [256,256,384] f32, Wq/Wk/Wv [384,64] f32
  q,k,v = x@W;  S = q@k^T * 384**-0.5; causal softmax; out = P@v  [256,256,64]

Sharding: data-parallel over batch B=256 -> 32 batches per core; weights
replicated.  All PE math in bf16 (inputs host-cast), fp32 PSUM accumulate.

Per core (NB=32 batches, DMA-grouped by G=8):

  1. x^T lands in SBUF bf16 via DMA-transpose straight from DRAM (one
     instruction per 128-channel chunk per group; group 0 split in two for
     faster rampup).  No PE transposes, no cast, no PSUM evacuation for x.
  2. Stacked projection [Wk|Wq]: one matmul chain -> kq^T [128,256] PSUM
     (rows 0:64 k^T, 64:128 q^T).  v in natural [token, 64] layout lands in
     the same PSUM tile; ONE [128,384] copy evacuates kq+v to SBUF.
  3. q^T moved to partitions 0:64 with an identity-shift matmul, evacuated
     into a [128,256] tile whose rows 64:128 are zero, so S^T runs K=128
     against the merged kq tile directly (q rows annihilate).
  4. S^T per key-chunk (fully-masked block skipped: 256+128 cols), single-op
     exp on ACT [128,384] PSUM->bf16, single-op causal mask on DVE
     (broadcast AP over both 128-col blocks).
  5. O computed NATURALLY (lhsT = P^T chunks, rhs = v chunks); softmax
     denominators via 3 extra N=1 matmuls against a static ones column.
     Per-partition reciprocal + single broadcast multiply -> bf16 out tile.
  6. One t-major store per group ([128, G*128] contiguous); host undoes the
     layout and upcasts to f32.
"""
import numpy as np

N_CORES = 8
B, T, C, H = 256, 256, 384, 64
NB = B // N_CORES          # 32 batches per core
G = 8                      # batches per DMA group
NG = NB // G               # 4 groups
SCALE = float(C) ** -0.5

_state = {}


def _build():
    import concourse.bacc as bacc
    import concourse.tile as tile
    import concourse.mybir as mybir
    from concourse.bass import AP
    from concourse.masks import make_identity, make_upper_triangular

    dt = mybir.dt
    f32 = dt.float32
    bf16 = dt.bfloat16
    AF = mybir.ActivationFunctionType

    nc = bacc.Bacc("TRN2", target_bir_lowering=False)
    x_d = nc.dram_tensor("x", [NB, T, C], bf16, kind="ExternalInput")
    w_d = nc.dram_tensor("W", [128, 576], bf16, kind="ExternalInput")
    out_d = nc.dram_tensor("out", [128, NB * 128], bf16, kind="ExternalOutput")

    with tile.TileContext(nc) as tc:
        with tc.tile_pool(name="setup", bufs=1) as setup, \
             tc.tile_pool(name="xin", bufs=1) as xin, \
             tc.tile_pool(name="og", bufs=3) as ogp, \
             tc.tile_pool(name="pa", bufs=3, space="PSUM") as pa, \
             tc.tile_pool(name="pb", bufs=3, space="PSUM") as pb, \
             tc.tile_pool(name="po", bufs=2, space="PSUM") as po, \
             tc.tile_pool(name="kvw", bufs=6) as kvw, \
             tc.tile_pool(name="qw", bufs=6) as qw, \
             tc.tile_pool(name="ptw", bufs=6) as ptw, \
             tc.tile_pool(name="rw", bufs=6) as rw:

            # --- one-time setup ---
            ident = setup.tile([128, 128], bf16)
            make_identity(nc, ident)
            bmask = setup.tile([128, 128], bf16)
            nc.gpsimd.memset(bmask, 0.0)
            nc.gpsimd.affine_select(
                out=bmask, in_=bmask,
                compare_op=mybir.AluOpType.is_ge,
                fill=-30000.0, base=0,
                pattern=[[1, 128]], channel_multiplier=-1)
            bmask2 = setup.tile([128, 256], bf16)
            nc.vector.tensor_copy(bmask2[:, 0:128], bmask)
            nc.vector.tensor_copy(bmask2[:, 128:256], bmask)
            ones1 = setup.tile([128, 1], bf16)
            nc.vector.memset(ones1, 1.0)
            w_s = setup.tile([128, 576], bf16)
            nc.sync.dma_start(out=w_s, in_=w_d[:, :])
            wkq_s = w_s[:, 0:384]
            wv_s = w_s[:, 384:576]

            # PE warm-up during initial DMA latency (p-state ramp is ~3us)
            warm = setup.tile([128, 128], bf16)
            nc.gpsimd.memset(warm, 0.0)
            wps = pa.tile([128, 384], f32, tag="pat")
            for _ in range(45):
                nc.tensor.matmul(wps[:, 0:128], warm, warm,
                                 start=True, stop=True)

            xts = []

            def load_group(g, nsplit=1):
                ts = [xin.tile([128, G * T], bf16, tag=f"xt{cc}", bufs=3,
                               name=f"xt{cc}") for cc in range(3)]
                bs = G // nsplit
                for s in range(nsplit):
                    for cc in range(3):
                        b0 = g * G + s * bs
                        src = x_d[b0:b0 + bs, :, cc * 128:(cc + 1) * 128]
                        src = src.rearrange("g t c -> (g t) c")
                        nc.sync.dma_start(
                            out=ts[cc][:, s * bs * T:(s + 1) * bs * T],
                            in_=src, transpose=True)
                return ts

            xts.append(load_group(0, nsplit=2))
            xts.append(load_group(1))

            # --- software-pipelined batch loop (stages skewed by batch
            # so every engine's in-order stream is dependency-ready) ---
            st_ = {}   # per-batch tile dict

            def stage_front(b):
                g, i = divmod(b, G)
                xtg = xts[g]
                xcol = i * T
                pat = pa.tile([128, 384], f32, tag="pat", name="pat")
                kv_s = kvw.tile([128, 384], bf16, tag="kv", name="kv_s")
                st_[b] = {"pat": pat, "kv": kv_s, "xtg": xtg, "xcol": xcol}
                kqps = pat[:, 0:256]
                vps = pat[:, 256:384]
                for cc in range(3):
                    nc.tensor.matmul(kqps,
                                     wkq_s[:, cc * 128:(cc + 1) * 128],
                                     xtg[cc][:, xcol:xcol + 256],
                                     start=(cc == 0), stop=(cc == 2))
                for tcx in range(2):
                    for cc in range(3):
                        nc.tensor.matmul(
                            vps[:, tcx * 64:(tcx + 1) * 64],
                            xtg[cc][:, xcol + tcx * 128:xcol + (tcx + 1) * 128],
                            wv_s[:, cc * 64:(cc + 1) * 64],
                            start=(cc == 0), stop=(cc == 2))
                nc.vector.tensor_copy(kv_s, pat)

            def stage_mid(b):
                s = st_[b]
                kv_s = s["kv"]
                q_s = qw.tile([64, 256], bf16, tag="qs", name="q_s")
                stps = pb.tile([128, 384], f32, tag="stps", name="stps")
                pt = ptw.tile([128, 384], bf16, tag="pt", name="pt")
                s.update(qs=q_s, stps=stps, pt=pt)
                pat = s["pat"]
                nc.scalar.copy(q_s[0:64, :], pat[64:128, 0:256])
                # one causal-bias pre-fill over both masked self-blocks,
                # then S^T (K=128; q rows of kv_s hit zero rows of q_s).
                # stps/pt layout: [self0 | self1 | cross]
                nc.tensor.matmul(stps[:, 0:256], ident, bmask2,
                                 start=True, stop=False)
                nc.tensor.matmul(stps[:, 0:128], kv_s[0:64, 0:128],
                                 q_s[:, 0:128], start=False, stop=True,
                                 skip_group_check=True)
                nc.tensor.matmul(stps[:, 128:256], kv_s[0:64, 128:256],
                                 q_s[:, 128:256], start=False, stop=True,
                                 skip_group_check=True)
                nc.tensor.matmul(stps[:, 256:384], kv_s[0:64, 0:128],
                                 q_s[:, 128:256], start=True, stop=True)
                nc.scalar.activation(pt, stps, AF.Exp, scale=SCALE)

            def stage_back(b):
                g, i = divmod(b, G)
                s = st_[b]
                kv_s, pt = s["kv"], s["pt"]
                pot = po.tile([128, 130], f32, tag="pot", name="pot")
                rec = rw.tile([128, 2], f32, tag="rec", name="rec")
                og = ogs[g]
                # NOTE: start=True clears has_written bank-wide, so each
                # accumulation group must run contiguously on this bank.
                nc.tensor.matmul(pot[:, 0:64], pt[:, 0:128],
                                 kv_s[:, 256:320], start=True, stop=True)
                nc.tensor.matmul(pot[:, 64:128], pt[:, 256:384],
                                 kv_s[:, 256:320], start=True, stop=False)
                nc.tensor.matmul(pot[:, 64:128], pt[:, 128:256],
                                 kv_s[:, 320:384], start=False, stop=True)
                nc.tensor.matmul(pot[:, 128:129], pt[:, 0:128], ones1,
                                 start=True, stop=True)
                nc.tensor.matmul(pot[:, 129:130], pt[:, 256:384], ones1,
                                 start=True, stop=False)
                nc.tensor.matmul(pot[:, 129:130], pt[:, 128:256], ones1,
                                 start=False, stop=True)
                nc.vector.reciprocal(rec, pot[:, 128:130])
                srcv = AP(pot.tensor, pot.offset,
                          [pot.ap[0], [64, 2], [1, 64]])
                rbc = AP(rec.tensor, rec.offset,
                         [rec.ap[0], [1, 2], [0, 64]])
                dst = og[:, i * 128:(i + 1) * 128]
                dst = AP(dst.tensor, dst.offset,
                         [dst.ap[0], [64, 2], [1, 64]])
                nc.vector.tensor_mul(dst, srcv, rbc)
                del st_[b]

            ogs = {}
            for ii in range(NB + 2):
                if ii < NB:
                    g = ii // G
                    if g not in ogs:
                        ogs[g] = ogp.tile([128, G * 128], bf16, tag="og",
                                          name="og")
                    stage_front(ii)
                if 1 <= ii <= NB:
                    stage_mid(ii - 1)
                if ii >= 2:
                    b2 = ii - 2
                    stage_back(b2)
                    g2, i2 = divmod(b2, G)
                    if i2 == G - 1:  # group complete -> store + prefetch
                        nc.sync.dma_start(
                            out=out_d[:, g2 * G * 128:(g2 + 1) * G * 128],
                            in_=ogs[g2])
                        if g2 + 2 < NG:
                            xts.append(load_group(g2 + 2))

    nc.finalize()
    return nc


def kernel(x, Wq, Wk, Wv, _trace=False):
    import ml_dtypes
    from concourse.bass_utils import run_bass_kernel_spmd

    if "nc" not in _state:
        _state["nc"] = _build()
    nc = _state["nc"]

    bf16 = ml_dtypes.bfloat16
    x = np.ascontiguousarray(np.asarray(x, dtype=np.float32)).astype(bf16)
    wkq = np.concatenate(
        [np.asarray(Wk, np.float32), np.asarray(Wq, np.float32)], axis=1)
    wkq = wkq.reshape(3, 128, 128).transpose(1, 0, 2).reshape(128, 384)
    wv = np.asarray(Wv, np.float32).reshape(3, 128, 64)
    wv = wv.transpose(1, 0, 2).reshape(128, 192)
    w = np.ascontiguousarray(
        np.concatenate([wkq, wv], axis=1)).astype(bf16)

    in_maps = [
        {"x": x[i * NB:(i + 1) * NB], "W": w}
        for i in range(N_CORES)
    ]
    res = run_bass_kernel_spmd(nc, in_maps, core_ids=list(range(N_CORES)),
                               trace=_trace)
    _state["exec_time_ns"] = res.exec_time_ns
    _state["trace"] = res.instructions_and_trace

    outs = []
    for i in range(N_CORES):
        o = np.asarray(res.results[i]["out"]).astype(np.float32)
        # [128, NB*128] t-major -> [NB, 256, 64]
        o = o.reshape(128, NB, 2, 64).transpose(1, 2, 0, 3).reshape(NB, T, H)
        outs.append(o)
    return np.concatenate(outs, axis=0)


# revision 35
# speedup vs baseline: 2.3583x; 1.0550x over previous
"""Causal single-head attention on 8 trn2 NeuronCores.

Problem (hardcoded): x [256,256,384] f32, Wq/Wk/Wv [384,64] f32
  q,k,v = x@W;  S = q@k^T * 384**-0.5; causal softmax; out = P@v  [256,256,64]

Sharding: data-parallel over batch B=256 -> 32 batches per core; weights
replicated.  All PE math in bf16 (inputs host-cast), fp32 PSUM accumulate.

Per core (NB=32 batches, DMA-grouped by G=8):

  1. x^T lands in SBUF bf16 via DMA-transpose straight from DRAM (one
     instruction per 128-channel chunk per group; group 0 split in two for
     faster rampup).  No PE transposes, no cast, no PSUM evacuation for x.
  2. Stacked projection [Wk|Wq]: one matmul chain -> kq^T [128,256] PSUM
     (rows 0:64 k^T, 64:128 q^T).  v in natural [token, 64] layout lands in
     the same PSUM tile; kq evacuates on ACT, v on DVE.
  3. q^T moved to partitions 0:64 with a partition-shifted gpsimd copy
     (SBUF->SBUF) out of the kq tile.
  4. S^T as [self0|self1|cross] (fully-masked block skipped: 384 cols);
     the causal mask is a -30000 additive bias pre-filled into the two
     self-blocks by ONE identity matmul, so a single ACT exp [128,384]
     produces the masked P^T with no separate mask op.
  5. O computed NATURALLY (lhsT = P^T chunks, rhs = v chunks); softmax
     denominators via 3 extra N=1 matmuls against a static ones column.
     Per-partition reciprocal + single broadcast multiply -> bf16 out tile.
  6. One t-major store per group ([128, G*128] contiguous); host undoes the
     layout and upcasts to f32.
"""
import numpy as np

N_CORES = 8
B, T, C, H = 256, 256, 384, 64
NB = B // N_CORES          # 32 batches per core
G = 8                      # batches per DMA group
NG = NB // G               # 4 groups
SCALE = float(C) ** -0.5

_state = {}


def _build():
    import concourse.bacc as bacc
    import concourse.tile as tile
    import concourse.mybir as mybir
    from concourse.bass import AP
    from concourse.masks import make_identity

    dt = mybir.dt
    f32 = dt.float32
    bf16 = dt.bfloat16
    AF = mybir.ActivationFunctionType

    nc = bacc.Bacc("TRN2", target_bir_lowering=False)
    x_d = nc.dram_tensor("x", [NB, T, C], bf16, kind="ExternalInput")
    w_d = nc.dram_tensor("W", [128, 576], bf16, kind="ExternalInput")
    out_d = nc.dram_tensor("out", [128, NB * 128], bf16, kind="ExternalOutput")

    with tile.TileContext(nc) as tc:
        with tc.tile_pool(name="setup", bufs=1) as setup, \
             tc.tile_pool(name="xin", bufs=1) as xin, \
             tc.tile_pool(name="og", bufs=3) as ogp, \
             tc.tile_pool(name="pa", bufs=4, space="PSUM") as pa, \
             tc.tile_pool(name="pb", bufs=2, space="PSUM") as pb, \
             tc.tile_pool(name="po", bufs=2, space="PSUM") as po, \
             tc.tile_pool(name="kvw", bufs=12) as kvw, \
             tc.tile_pool(name="qw", bufs=12) as qw, \
             tc.tile_pool(name="ptw", bufs=12) as ptw, \
             tc.tile_pool(name="rw", bufs=12) as rw:

            # --- one-time setup ---
            ident = setup.tile([128, 128], bf16)
            make_identity(nc, ident)
            bmask = setup.tile([128, 128], bf16)
            nc.gpsimd.memset(bmask, 0.0)
            nc.gpsimd.affine_select(
                out=bmask, in_=bmask,
                compare_op=mybir.AluOpType.is_ge,
                fill=-30000.0, base=0,
                pattern=[[1, 128]], channel_multiplier=-1)
            bmask2 = setup.tile([128, 256], bf16)
            nc.vector.tensor_copy(bmask2[:, 0:128], bmask)
            nc.vector.tensor_copy(bmask2[:, 128:256], bmask)
            ones1 = setup.tile([128, 1], bf16)
            nc.vector.memset(ones1, 1.0)
            w_s = setup.tile([128, 576], bf16)
            nc.sync.dma_start(out=w_s, in_=w_d[:, :])
            wkq_s = w_s[:, 0:384]
            wv_s = w_s[:, 384:576]

            # PE warm-up during initial DMA latency (p-state ramp is ~3us)
            warm = setup.tile([128, 128], bf16)
            nc.gpsimd.memset(warm, 0.0)
            wps = pa.tile([128, 384], f32, tag="pat")
            for _ in range(35):
                nc.tensor.matmul(wps[:, 0:128], warm, warm,
                                 start=True, stop=True)

            xts = []

            def load_group(g, nsplit=1):
                ts = [xin.tile([128, G * T], bf16, tag=f"xt{cc}", bufs=3,
                               name=f"xt{cc}") for cc in range(3)]
                bs = G // nsplit
                for s in range(nsplit):
                    for cc in range(3):
                        b0 = g * G + s * bs
                        src = x_d[b0:b0 + bs, :, cc * 128:(cc + 1) * 128]
                        src = src.rearrange("g t c -> (g t) c")
                        nc.sync.dma_start(
                            out=ts[cc][:, s * bs * T:(s + 1) * bs * T],
                            in_=src, transpose=True)
                return ts

            xts.append(load_group(0, nsplit=2))
            xts.append(load_group(1, nsplit=2))

            # --- software-pipelined batch loop (stages skewed by batch
            # so every engine's in-order stream is dependency-ready) ---
            st_ = {}   # per-batch tile dict

            def stage_front(b):
                g, i = divmod(b, G)
                xtg = xts[g]
                xcol = i * T
                pat = pa.tile([128, 384], f32, tag="pat", name="pat")
                kv_s = kvw.tile([128, 384], bf16, tag="kv", name="kv_s")
                st_[b] = {"pat": pat, "kv": kv_s, "xtg": xtg, "xcol": xcol}
                kqps = pat[:, 0:256]
                vps = pat[:, 256:384]
                for cc in range(3):
                    nc.tensor.matmul(kqps,
                                     wkq_s[:, cc * 128:(cc + 1) * 128],
                                     xtg[cc][:, xcol:xcol + 256],
                                     start=(cc == 0), stop=(cc == 2))
                for tcx in range(2):
                    for cc in range(3):
                        nc.tensor.matmul(
                            vps[:, tcx * 64:(tcx + 1) * 64],
                            xtg[cc][:, xcol + tcx * 128:xcol + (tcx + 1) * 128],
                            wv_s[:, cc * 64:(cc + 1) * 64],
                            start=(cc == 0), stop=(cc == 2))
                nc.scalar.copy(kv_s[:, 0:256], kqps)
                nc.vector.tensor_copy(kv_s[:, 256:384], vps)

            def stage_mid(b):
                s = st_[b]
                kv_s = s["kv"]
                q_s = qw.tile([64, 256], bf16, tag="qs", name="q_s")
                stps = pb.tile([128, 384], f32, tag="stps", name="stps")
                pt = ptw.tile([128, 384], bf16, tag="pt", name="pt")
                s.update(qs=q_s, stps=stps, pt=pt)
                nc.gpsimd.tensor_copy(q_s[0:64, :], kv_s[64:128, 0:256])
                # one causal-bias pre-fill over both masked self-blocks,
                # then S^T (K=128; q rows of kv_s hit zero rows of q_s).
                # stps/pt layout: [self0 | self1 | cross]
                nc.tensor.matmul(stps[:, 0:256], ident, bmask2,
                                 start=True, stop=False)
                nc.tensor.matmul(stps[:, 0:128], kv_s[0:64, 0:128],
                                 q_s[:, 0:128], start=False, stop=True,
                                 skip_group_check=True)
                nc.tensor.matmul(stps[:, 128:256], kv_s[0:64, 128:256],
                                 q_s[:, 128:256], start=False, stop=True,
                                 skip_group_check=True)
                nc.tensor.matmul(stps[:, 256:384], kv_s[0:64, 0:128],
                                 q_s[:, 128:256], start=True, stop=True)
                nc.scalar.activation(pt, stps, AF.Exp, scale=SCALE)

            def stage_back(b):
                g, i = divmod(b, G)
                s = st_[b]
                kv_s, pt = s["kv"], s["pt"]
                pot = po.tile([128, 130], f32, tag="pot", name="pot")
                rec = rw.tile([128, 2], f32, tag="rec", name="rec")
                og = ogs[g]
                # NOTE: start=True clears has_written bank-wide, so each
                # accumulation group must run contiguously on this bank.
                nc.tensor.matmul(pot[:, 0:64], pt[:, 0:128],
                                 kv_s[:, 256:320], start=True, stop=True)
                nc.tensor.matmul(pot[:, 64:128], pt[:, 256:384],
                                 kv_s[:, 256:320], start=True, stop=False)
                nc.tensor.matmul(pot[:, 64:128], pt[:, 128:256],
                                 kv_s[:, 320:384], start=False, stop=True)
                nc.tensor.matmul(pot[:, 128:129], pt[:, 0:128], ones1,
                                 start=True, stop=True)
                nc.tensor.matmul(pot[:, 129:130], pt[:, 256:384], ones1,
                                 start=True, stop=False)
                nc.tensor.matmul(pot[:, 129:130], pt[:, 128:256], ones1,
                                 start=False, stop=True)
                nc.vector.reciprocal(rec, pot[:, 128:130])
                srcv = AP(pot.tensor, pot.offset,
                          [pot.ap[0], [64, 2], [1, 64]])
                rbc = AP(rec.tensor, rec.offset,
                         [rec.ap[0], [1, 2], [0, 64]])
                dst = og[:, i * 128:(i + 1) * 128]
                dst = AP(dst.tensor, dst.offset,
                         [dst.ap[0], [64, 2], [1, 64]])
                nc.vector.tensor_mul(dst, srcv, rbc)
                del st_[b]

            ogs = {}
            for ii in range(NB + 2):
                if ii < NB:
                    g = ii // G
                    if g not in ogs:
                        ogs[g] = ogp.tile([128, G * 128], bf16, tag="og",
                                          name="og")
                    stage_front(ii)
                if 1 <= ii <= NB:
                    stage_mid(ii - 1)
                if ii >= 2:
                    b2 = ii - 2
                    stage_back(b2)
                    g2, i2 = divmod(b2, G)
                    if g2 == NG - 1 and i2 == G // 2 - 1:
                        nc.sync.dma_start(
                            out=out_d[:, g2 * G * 128:g2 * G * 128 + G * 64],
                            in_=ogs[g2][:, 0:G * 64])
                    if i2 == G - 1:  # group complete -> store + prefetch
                        if g2 == NG - 1:
                            nc.sync.dma_start(
                                out=out_d[:, g2 * G * 128 + G * 64:
                                          (g2 + 1) * G * 128],
                                in_=ogs[g2][:, G * 64:G * 128])
                        else:
                            nc.sync.dma_start(
                                out=out_d[:, g2 * G * 128:(g2 + 1) * G * 128],
                                in_=ogs[g2])
                        if g2 + 2 < NG:
                            xts.append(load_group(g2 + 2, nsplit=2))

    nc.finalize()
    return nc


def kernel(x, Wq, Wk, Wv, _trace=False):
    import ml_dtypes
    from concourse.bass_utils import run_bass_kernel_spmd

    if "nc" not in _state:
        _state["nc"] = _build()
    nc = _state["nc"]

    bf16 = ml_dtypes.bfloat16
    x = np.ascontiguousarray(np.asarray(x, dtype=np.float32)).astype(bf16)
    wkq = np.concatenate(
        [np.asarray(Wk, np.float32), np.asarray(Wq, np.float32)], axis=1)
    wkq = wkq.reshape(3, 128, 128).transpose(1, 0, 2).reshape(128, 384)
    wv = np.asarray(Wv, np.float32).reshape(3, 128, 64)
    wv = wv.transpose(1, 0, 2).reshape(128, 192)
    w = np.ascontiguousarray(
        np.concatenate([wkq, wv], axis=1)).astype(bf16)

    in_maps = [
        {"x": x[i * NB:(i + 1) * NB], "W": w}
        for i in range(N_CORES)
    ]
    res = run_bass_kernel_spmd(nc, in_maps, core_ids=list(range(N_CORES)),
                               trace=_trace)
    _state["exec_time_ns"] = res.exec_time_ns
    _state["trace"] = res.instructions_and_trace

    outs = []
    for i in range(N_CORES):
        o = np.asarray(res.results[i]["out"]).astype(np.float32)
        # [128, NB*128] t-major -> [NB, 256, 64]
        o = o.reshape(128, NB, 2, 64).transpose(1, 2, 0, 3).reshape(NB, T, H)
        outs.append(o)
    return np.concatenate(outs, axis=0)


# revision 42
# speedup vs baseline: 2.3866x; 1.0120x over previous
"""Causal single-head attention on 8 trn2 NeuronCores.

Problem (hardcoded): x [256,256,384] f32, Wq/Wk/Wv [384,64] f32
  q,k,v = x@W;  S = q@k^T * 384**-0.5; causal softmax; out = P@v  [256,256,64]

Sharding: data-parallel over batch B=256 -> 32 batches per core; weights
replicated.  All PE math in bf16 (inputs host-cast), fp32 PSUM accumulate.

Per core (NB=32 batches, DMA-grouped by G=8):

  1. x^T lands in SBUF bf16 via DMA-transpose straight from DRAM (one
     instruction per 128-channel chunk per group; group 0 split [2,6] so
     batch 0's data lands ~4us earlier).  No PE transposes, no cast, no
     PSUM evacuation for x.
  2. Stacked projection [Wk|Wq]: one matmul chain -> kq^T [128,256] PSUM
     (rows 0:64 k^T, 64:128 q^T).  v in natural [token, 64] layout lands in
     the same PSUM tile; kq evacuates on DVE, v on ACT.
  3. q^T moved to partitions 0:64 with a partition-shifted gpsimd copy
     (SBUF->SBUF) out of the kq tile.
  4. S^T as [self0|self1|cross] (fully-masked block skipped: 384 cols);
     the causal mask is a -30000 additive bias pre-filled into the two
     self-blocks by ONE identity matmul, so a single ACT exp [128,384]
     produces the masked P^T with no separate mask op.
  5. O computed NATURALLY (lhsT = P^T chunks, rhs = v chunks); softmax
     denominators via 3 N=1 matmuls against a static ones column, issued
     BEFORE the O matmuls so the reciprocal overlaps them.  PSUM matmul
     accumulation groups are kept contiguous per bank (start=True clears
     has_written bank-wide on real HW).
     Per-partition reciprocal + single broadcast multiply -> bf16 out tile.
  6. One t-major store per group ([128, G*128] contiguous); host undoes the
     layout and upcasts to f32.
"""
import numpy as np

N_CORES = 8
B, T, C, H = 256, 256, 384, 64
NB = B // N_CORES          # 32 batches per core
G = 8                      # batches per DMA group
NG = NB // G               # 4 groups
SCALE = float(C) ** -0.5

_state = {}


def _build():
    import concourse.bacc as bacc
    import concourse.tile as tile
    import concourse.mybir as mybir
    from concourse.bass import AP
    from concourse.masks import make_identity

    dt = mybir.dt
    f32 = dt.float32
    bf16 = dt.bfloat16
    AF = mybir.ActivationFunctionType

    nc = bacc.Bacc("TRN2", target_bir_lowering=False)
    x_d = nc.dram_tensor("x", [NB, T, C], bf16, kind="ExternalInput")
    w_d = nc.dram_tensor("W", [128, 576], bf16, kind="ExternalInput")
    out_d = nc.dram_tensor("out", [128, NB * 128], bf16, kind="ExternalOutput")

    with tile.TileContext(nc) as tc:
        with tc.tile_pool(name="setup", bufs=1) as setup, \
             tc.tile_pool(name="xin", bufs=1) as xin, \
             tc.tile_pool(name="og", bufs=3) as ogp, \
             tc.tile_pool(name="pa", bufs=3, space="PSUM") as pa, \
             tc.tile_pool(name="pb", bufs=3, space="PSUM") as pb, \
             tc.tile_pool(name="po", bufs=2, space="PSUM") as po, \
             tc.tile_pool(name="kvw", bufs=12) as kvw, \
             tc.tile_pool(name="qw", bufs=12) as qw, \
             tc.tile_pool(name="ptw", bufs=12) as ptw, \
             tc.tile_pool(name="rw", bufs=12) as rw:

            # --- one-time setup ---
            ident = setup.tile([128, 128], bf16)
            make_identity(nc, ident)
            bmask = setup.tile([128, 128], bf16)
            nc.gpsimd.memset(bmask, 0.0)
            nc.gpsimd.affine_select(
                out=bmask, in_=bmask,
                compare_op=mybir.AluOpType.is_ge,
                fill=-30000.0, base=0,
                pattern=[[1, 128]], channel_multiplier=-1)
            bmask2 = setup.tile([128, 256], bf16)
            nc.vector.tensor_copy(bmask2[:, 0:128], bmask)
            nc.vector.tensor_copy(bmask2[:, 128:256], bmask)
            ones1 = setup.tile([128, 1], bf16)
            nc.vector.memset(ones1, 1.0)
            w_s = setup.tile([128, 576], bf16)
            nc.sync.dma_start(out=w_s, in_=w_d[:, :])
            wkq_s = w_s[:, 0:384]
            wv_s = w_s[:, 384:576]

            # PE warm-up during initial DMA latency (p-state ramp is ~3us)
            warm = setup.tile([128, 128], bf16)
            nc.gpsimd.memset(warm, 0.0)
            wps = pa.tile([128, 384], f32, tag="pat")
            for _ in range(40):
                nc.tensor.matmul(wps[:, 0:128], warm, warm,
                                 start=True, stop=True)

            xts = []

            def load_group(g, nsplit=1, subs=None):
                ts = [xin.tile([128, G * T], bf16, tag=f"xt{cc}", bufs=3,
                               name=f"xt{cc}") for cc in range(3)]
                if subs is None:
                    bs = G // nsplit
                    subs = [bs] * nsplit
                off = 0
                for bs in subs:
                    for cc in range(3):
                        b0 = g * G + off
                        src = x_d[b0:b0 + bs, :, cc * 128:(cc + 1) * 128]
                        src = src.rearrange("g t c -> (g t) c")
                        nc.sync.dma_start(
                            out=ts[cc][:, off * T:(off + bs) * T],
                            in_=src, transpose=True)
                    off += bs
                return ts

            xts.append(load_group(0, subs=[2, 6]))
            xts.append(load_group(1, nsplit=2))

            # --- software-pipelined batch loop (stages skewed by batch
            # so every engine's in-order stream is dependency-ready) ---
            st_ = {}   # per-batch tile dict

            def stage_front(b):
                g, i = divmod(b, G)
                xtg = xts[g]
                xcol = i * T
                pat = pa.tile([128, 384], f32, tag="pat", name="pat")
                kv_s = kvw.tile([128, 384], bf16, tag="kv", name="kv_s")
                st_[b] = {"pat": pat, "kv": kv_s, "xtg": xtg, "xcol": xcol}
                kqps = pat[:, 0:256]
                vps = pat[:, 256:384]
                for cc in range(3):
                    nc.tensor.matmul(kqps,
                                     wkq_s[:, cc * 128:(cc + 1) * 128],
                                     xtg[cc][:, xcol:xcol + 256],
                                     start=(cc == 0), stop=(cc == 2))
                for tcx in range(2):
                    for cc in range(3):
                        nc.tensor.matmul(
                            vps[:, tcx * 64:(tcx + 1) * 64],
                            xtg[cc][:, xcol + tcx * 128:xcol + (tcx + 1) * 128],
                            wv_s[:, cc * 64:(cc + 1) * 64],
                            start=(cc == 0), stop=(cc == 2))
                nc.vector.tensor_copy(kv_s[:, 0:256], kqps)
                nc.scalar.copy(kv_s[:, 256:384], vps)
                stps = pb.tile([128, 384], f32, tag="stps", name="stps")
                st_[b]["stps"] = stps
                nc.tensor.matmul(stps[:, 0:256], ident, bmask2,
                                 start=True, stop=False)

            def stage_mid(b):
                s = st_[b]
                kv_s = s["kv"]
                q_s = qw.tile([64, 256], bf16, tag="qs", name="q_s")
                stps = s["stps"]
                pt = ptw.tile([128, 384], bf16, tag="pt", name="pt")
                s.update(qs=q_s, pt=pt)
                nc.gpsimd.tensor_copy(q_s[0:64, :], kv_s[64:128, 0:256])
                nc.tensor.matmul(stps[:, 0:128], kv_s[0:64, 0:128],
                                 q_s[:, 0:128], start=False, stop=True,
                                 skip_group_check=True)
                nc.tensor.matmul(stps[:, 128:256], kv_s[0:64, 128:256],
                                 q_s[:, 128:256], start=False, stop=True,
                                 skip_group_check=True)
                nc.tensor.matmul(stps[:, 256:384], kv_s[0:64, 0:128],
                                 q_s[:, 128:256], start=True, stop=True)
                nc.scalar.activation(pt, stps, AF.Exp, scale=SCALE)

            def stage_back(b):
                g, i = divmod(b, G)
                s = st_[b]
                kv_s, pt = s["kv"], s["pt"]
                pot = po.tile([128, 130], f32, tag="pot", name="pot")
                rec = rw.tile([128, 2], f32, tag="rec", name="rec")
                og = ogs[g]
                # NOTE: start=True clears has_written bank-wide, so each
                # accumulation group must run contiguously on this bank.
                nc.tensor.matmul(pot[:, 128:129], pt[:, 0:128], ones1,
                                 start=True, stop=True)
                nc.tensor.matmul(pot[:, 129:130], pt[:, 256:384], ones1,
                                 start=True, stop=False)
                nc.tensor.matmul(pot[:, 129:130], pt[:, 128:256], ones1,
                                 start=False, stop=True)
                nc.tensor.matmul(pot[:, 0:64], pt[:, 0:128],
                                 kv_s[:, 256:320], start=True, stop=True)
                nc.tensor.matmul(pot[:, 64:128], pt[:, 256:384],
                                 kv_s[:, 256:320], start=True, stop=False)
                nc.tensor.matmul(pot[:, 64:128], pt[:, 128:256],
                                 kv_s[:, 320:384], start=False, stop=True)
                nc.vector.reciprocal(rec, pot[:, 128:130])
                srcv = AP(pot.tensor, pot.offset,
                          [pot.ap[0], [64, 2], [1, 64]])
                rbc = AP(rec.tensor, rec.offset,
                         [rec.ap[0], [1, 2], [0, 64]])
                dst = og[:, i * 128:(i + 1) * 128]
                dst = AP(dst.tensor, dst.offset,
                         [dst.ap[0], [64, 2], [1, 64]])
                nc.vector.tensor_mul(dst, srcv, rbc)
                del st_[b]

            ogs = {}
            for ii in range(NB + 2):
                if ii < NB:
                    g = ii // G
                    if g not in ogs:
                        ogs[g] = ogp.tile([128, G * 128], bf16, tag="og",
                                          name="og")
                    stage_front(ii)
                if 1 <= ii <= NB:
                    stage_mid(ii - 1)
                if ii >= 2:
                    b2 = ii - 2
                    stage_back(b2)
                    g2, i2 = divmod(b2, G)
                    if g2 == NG - 1 and i2 == G // 2 - 1:
                        nc.sync.dma_start(
                            out=out_d[:, g2 * G * 128:g2 * G * 128 + G * 64],
                            in_=ogs[g2][:, 0:G * 64])
                    if i2 == G - 1:  # group complete -> store + prefetch
                        if g2 == NG - 1:
                            nc.sync.dma_start(
                                out=out_d[:, g2 * G * 128 + G * 64:
                                          (g2 + 1) * G * 128],
                                in_=ogs[g2][:, G * 64:G * 128])
                        else:
                            nc.sync.dma_start(
                                out=out_d[:, g2 * G * 128:(g2 + 1) * G * 128],
                                in_=ogs[g2])
                        if g2 + 2 < NG:
                            xts.append(load_group(g2 + 2, nsplit=2))

    nc.finalize()
    return nc


def kernel(x, Wq, Wk, Wv, _trace=False):
    import ml_dtypes
    from concourse.bass_utils import run_bass_kernel_spmd

    if "nc" not in _state:
        _state["nc"] = _build()
    nc = _state["nc"]

    bf16 = ml_dtypes.bfloat16
    x = np.ascontiguousarray(np.asarray(x, dtype=np.float32)).astype(bf16)
    wkq = np.concatenate(
        [np.asarray(Wk, np.float32), np.asarray(Wq, np.float32)], axis=1)
    wkq = wkq.reshape(3, 128, 128).transpose(1, 0, 2).reshape(128, 384)
    wv = np.asarray(Wv, np.float32).reshape(3, 128, 64)
    wv = wv.transpose(1, 0, 2).reshape(128, 192)
    w = np.ascontiguousarray(
        np.concatenate([wkq, wv], axis=1)).astype(bf16)

    in_maps = [
        {"x": x[i * NB:(i + 1) * NB], "W": w}
        for i in range(N_CORES)
    ]
    res = run_bass_kernel_spmd(nc, in_maps, core_ids=list(range(N_CORES)),
                               trace=_trace)
    _state["exec_time_ns"] = res.exec_time_ns
    _state["trace"] = res.instructions_and_trace

    outs = []
    for i in range(N_CORES):
        o = np.asarray(res.results[i]["out"]).astype(np.float32)
        # [128, NB*128] t-major -> [NB, 256, 64]
        o = o.reshape(128, NB, 2, 64).transpose(1, 2, 0, 3).reshape(NB, T, H)
        outs.append(o)
    return np.concatenate(outs, axis=0)
